# revision 7
# baseline (speedup 1.0000x reference)
"""Trainium2 Bass kernel for nn_AttentionModule (multi-head attention pooling).

Math per sample n (N=16384, SPLIT=100, INPUT_DIM=128, H=4 heads, PER_DIM=64):
  xs = x.reshape(n, 100, 128)
  h[s, (hd,o)] = xs[s, :] @ W[hd][:, o]          (projection, out 256 cols)
  score[s, hd] = leaky_relu(h[s, hd*64:] . q[hd], 0.2) = xs[s,:] . wq[hd]
  att = softmax_s(score);  out[n] = sum_s att[s,hd] * h[s, (hd,o)]

Strategy (data-parallel over 8 cores, 2048 samples each), engine-balanced:
  - x block [128 samples, 12800] loaded with a CASTING gpsimd (SWDGE) DMA:
    bf16 lands in SBUF directly, no engine cast pass.
  - per split s: PE-transpose x slice [n, i] -> [i, n] (PSUM bank tail),
    ACT evicts it to SBUF (ACT also stages Wb so matmuls keep ONE sync
    wait), then one matmul with the transposed slice stationary and
    Wb = [W | wq] bf16 [128, 260] moving -> h+score [n=128p, 260] fp32 PSUM.
  - h evict (fp32 PSUM -> bf16 SBUF, layout [n, c, s]) on ACT; score evict
    (fp32 [n, 4, s]) on DVE.
  - the block's 100 splits are processed as two independent halves (48+52)
    with PER-HALF h/score/u tiles, so the DVE pooling chain of half k runs
    fully parallel to ACT's evicts of half k+1 (no false tile deps).
  - softmax without max subtraction (scores are O(6), exp safe in f32).
  - pooling on DVE: tensor_tensor multiply with stride-0 broadcast of u over
    the 64 per-head dims (runs 2x_1P), then a pairwise bf16 fold tree down to
    width 3 (tensor_tensor keeps the DVE 2x mode; tensor_reduce runs 1x) and
    one small reduce; normalize by 1/sum at the end.
"""

import sys

if "/opt/trn_rl_repo" not in sys.path:
    sys.path.insert(0, "/opt/trn_rl_repo")

import numpy as np

N_TOTAL = 16384
NCORES = 8
S = 100
I = 128
H = 4
O = 64
OUT = 256
COLS = 260  # 256 projection cols + 4 score cols
BLK = 128
SG = 4  # splits per PSUM group; transposes live in each bank's free tail
HALF = 48  # first-half split count (must be a multiple of SG)

# --- engine balance tunables ---
G_D = 0  # xt-evict groups handled by DVE (rest by ACT)
# how to cast x fp32->bf16: "dma" = casting SWDGE DMA (slow, ~90GB/s),
# "gpsimd" = HWDGE load + GPSIMD tensor_copy, "split" = HWDGE + DVE/ACT halves
CAST_MODE = "gpsimd"

_BUILT = {}


def build_bass(npc):
    """Build the per-core Bass program for npc samples (npc % 128 == 0)."""
    import concourse.bass as bass
    import concourse.mybir as mybir
    from concourse import bacc
    from concourse.bass import broadcast_tensor_aps
    from concourse.masks import make_identity
    from concourse.tile import TileContext

    dt = mybir.dt
    nblk = npc // BLK
    nc = bacc.Bacc()

    xd = nc.declare_dram_parameter("x", [npc, S * I], dt.float32, isOutput=False)
    Wd = nc.declare_dram_parameter("W", [H, I, O], dt.float32, isOutput=False)
    qd = nc.declare_dram_parameter("q", [H, O], dt.float32, isOutput=False)
    od = nc.declare_dram_parameter("out", [npc, OUT], dt.float32, isOutput=True)

    groups = []
    s0 = 0
    while s0 < S:
        groups.append((s0, min(SG, S - s0)))
        s0 += SG
    halves = ((0, HALF), (HALF, S))

    with TileContext(nc) as tc:
        with (
            tc.tile_pool(name="const", bufs=1) as cpool,
            tc.tile_pool(name="xbfp", bufs=2) as xbfp,
            tc.tile_pool(name="xpool", bufs=2) as xpool,
            tc.tile_pool(name="xtp", bufs=3) as xtp,
            tc.tile_pool(name="hwp", bufs=2) as hwp,
            tc.tile_pool(name="smp", bufs=2) as smp,
            tc.tile_pool(name="outp", bufs=2) as outp,
            tc.tile_pool(name="redp", bufs=2) as redp,
            tc.tile_pool(name="php", bufs=2, space="PSUM") as php,
        ):
            # ---- setup: identity (for PE transpose) and Wb = [W | wq] bf16
            # PE instructions only support ONE sync wait (walrus S3_LW limit),
            # so every tensor a PE matmul reads is staged through the SAME
            # engine that produced its other SBUF operand: Wb exists twice,
            # once DVE-staged and once ACT-staged.
            ident_s = cpool.tile([128, 128], dt.bfloat16)
            make_identity(nc, ident_s[:, :])
            ident = cpool.tile([128, 128], dt.bfloat16)
            nc.vector.tensor_copy(out=ident[:, :], in_=ident_s[:, :])

            Wf = cpool.tile([128, H, O], dt.float32)  # [i, hd, o]
            nc.sync.dma_start(out=Wf[:, :, :], in_=Wd[:, :, :].rearrange("h i o -> i h o"))
            W2s = cpool.tile([O, H, I], dt.float32)  # [o, hd, i]
            nc.sync.dma_start(out=W2s[:, :, :], in_=Wd[:, :, :].rearrange("h i o -> o h i"))
            q2s = cpool.tile([O, H], dt.float32)  # [o, hd]
            nc.sync.dma_start(out=q2s[:, :], in_=qd[:, :].rearrange("h o -> o h"))
            W2 = cpool.tile([O, H, I], dt.float32)
            nc.vector.tensor_copy(out=W2[:, :, :], in_=W2s[:, :, :])
            q2 = cpool.tile([O, H], dt.float32)
            nc.vector.tensor_copy(out=q2[:, :], in_=q2s[:, :])

            wqp = php.tile([128, H], dt.float32, tag="ph")
            for hd in range(H):
                # wq[:, hd] -- lhsT=[o,(i)], rhs=[o,1] -> out [i,1]
                nc.tensor.matmul(
                    wqp[:, hd : hd + 1],
                    lhsT=W2[:, hd, :],
                    rhs=q2[:, hd : hd + 1],
                    start=True,
                    stop=True,
                )
            Wb = cpool.tile([128, COLS], dt.bfloat16)  # DVE-staged
            nc.vector.tensor_copy(
                out=Wb[:, 0:OUT].rearrange("p (h o) -> p h o", h=H), in_=Wf[:, :, :]
            )
            nc.vector.tensor_copy(out=Wb[:, OUT:COLS], in_=wqp[:, :])
            Wb_a = cpool.tile([128, COLS], dt.bfloat16)  # ACT-staged
            nc.scalar.copy(out=Wb_a[:, :], in_=Wb[:, :])

            for b in range(nblk):
                # ---- load x block, cast to bf16
                if CAST_MODE == "dma":
                    xbfa = xbfp.tile([128, 6400], dt.bfloat16, tag="xbf")
                    nc.gpsimd.dma_start(
                        out=xbfa[:, :], in_=xd[b * BLK : (b + 1) * BLK, 0:6400]
                    )
                    xbfb = xbfp.tile([128, 6400], dt.bfloat16, tag="xbf")
                    nc.gpsimd.dma_start(
                        out=xbfb[:, :], in_=xd[b * BLK : (b + 1) * BLK, 6400:12800]
                    )
                else:
                    xa = xpool.tile([128, 6400], dt.float32, tag="x")
                    nc.sync.dma_start(
                        out=xa[:, :], in_=xd[b * BLK : (b + 1) * BLK, 0:6400]
                    )
                    xb2 = xpool.tile([128, 6400], dt.float32, tag="x")
                    nc.sync.dma_start(
                        out=xb2[:, :], in_=xd[b * BLK : (b + 1) * BLK, 6400:12800]
                    )
                    xbfa = xbfp.tile([128, 6400], dt.bfloat16, tag="xbf")
                    xbfb = xbfp.tile([128, 6400], dt.bfloat16, tag="xbf")
                    if CAST_MODE == "gpsimd":
                        nc.gpsimd.tensor_copy(out=xbfa[:, :], in_=xa[:, :])
                        nc.gpsimd.tensor_copy(out=xbfb[:, :], in_=xb2[:, :])
                    else:
                        nc.vector.tensor_copy(out=xbfa[:, :], in_=xa[:, :])
                        nc.scalar.copy(out=xbfb[:, :], in_=xb2[:, :])
                xhalves = (xbfa, xbfb)

                # per-half tiles: half k's pooling chain is independent of
                # half k+1's evicts
                hw = [
                    hwp.tile([128, OUT, hi - lo], dt.bfloat16, tag=f"hw{k}", name=f"hw{k}")
                    for k, (lo, hi) in enumerate(halves)
                ]
                scs = [
                    smp.tile([128, H, hi - lo], dt.float32, tag=f"sc{k}", name=f"sc{k}")
                    for k, (lo, hi) in enumerate(halves)
                ]
                t1 = [
                    smp.tile([128, H, hi - lo], dt.float32, tag=f"t1{k}", name=f"t1{k}")
                    for k, (lo, hi) in enumerate(halves)
                ]
                uf = [
                    smp.tile([128, H, hi - lo], dt.float32, tag=f"uf{k}", name=f"uf{k}")
                    for k, (lo, hi) in enumerate(halves)
                ]
                ub = [
                    smp.tile([128, H, hi - lo], dt.bfloat16, tag=f"ub{k}", name=f"ub{k}")
                    for k, (lo, hi) in enumerate(halves)
                ]
                prh = [
                    redp.tile([128, OUT], dt.float32, tag=f"prh{k}", name=f"prh{k}")
                    for k in range(2)
                ]

                def tail_half(k):
                    # u = exp(leaky(score)); scale h by u and fold pairwise
                    # down to width 3, then one small reduce.
                    lo, hi = halves[k]
                    w = hi - lo
                    hwk, sck, t1k, ufk, ubk = hw[k], scs[k], t1[k], uf[k], ub[k]
                    nc.vector.tensor_scalar_mul(t1k[:, :, :], sck[:, :, :], 0.2)
                    nc.vector.tensor_tensor(
                        out=sck[:, :, :],
                        in0=sck[:, :, :],
                        in1=t1k[:, :, :],
                        op=mybir.AluOpType.max,
                    )
                    nc.scalar.activation(
                        out=ufk[:, :, :],
                        in_=sck[:, :, :],
                        func=mybir.ActivationFunctionType.Exp,
                    )
                    nc.vector.tensor_copy(out=ubk[:, :, :], in_=ufk[:, :, :])
                    in0 = hwk[:, :, :].rearrange("p (h o) s -> p h o s", h=H)
                    in1 = ubk[:, :, :].unsqueeze(2)
                    in0b, in1b = broadcast_tensor_aps(in0, in1)
                    nc.vector.tensor_tensor(
                        out=in0b, in0=in0b, in1=in1b, op=mybir.AluOpType.mult
                    )
                    # pairwise bf16 fold tree down to width 3 (stays in DVE 2x
                    # mode; tensor_reduce runs 1x so keep its width tiny).
                    leftovers = []
                    lf = 0
                    with nc.allow_low_precision("bf16 pairwise partial sums"):
                        while w > 3:
                            a2 = w // 2
                            nc.vector.tensor_tensor(
                                out=hwk[:, :, lf : lf + a2],
                                in0=hwk[:, :, lf : lf + a2],
                                in1=hwk[:, :, lf + a2 : lf + 2 * a2],
                                op=mybir.AluOpType.add,
                            )
                            if w % 2:
                                leftovers.append(lf + 2 * a2)
                            w = a2
                    nc.vector.tensor_reduce(
                        out=prh[k][:, :],
                        in_=hwk[:, :, lf : lf + w],
                        axis=mybir.AxisListType.X,
                        op=mybir.AluOpType.add,
                    )
                    for col in leftovers:
                        nc.vector.tensor_tensor(
                            out=prh[k][:, :],
                            in0=prh[k][:, :],
                            in1=hwk[:, :, col],
                            op=mybir.AluOpType.add,
                        )

                for gi, (s0, ns) in enumerate(groups):
                    k = 0 if s0 < HALF else 1
                    lo = halves[k][0]
                    xt = xtp.tile([128, SG, 128], dt.bfloat16, tag="xt")
                    ph = php.tile([128, SG, 512], dt.float32, tag="ph")
                    # transposes land in each bank's free tail (cols 448:512
                    # as fp32 = 128 bf16), so no separate PSUM pool is needed
                    for j in range(ns):
                        s = s0 + j
                        hv, off = (0, s) if s < 50 else (1, s - 50)
                        src = xhalves[hv][:, off * 128 : (off + 1) * 128]
                        nc.tensor.transpose(
                            ph[:, j, 448:512].bitcast(dt.bfloat16), src, ident[:, :]
                        )
                    if gi < G_D:
                        nc.vector.tensor_copy(
                            out=xt[:, 0:ns, :],
                            in_=ph[:, 0:ns, 448:512].bitcast(dt.bfloat16),
                        )
                        wmov = Wb
                    else:
                        nc.scalar.copy(
                            out=xt[:, 0:ns, :],
                            in_=ph[:, 0:ns, 448:512].bitcast(dt.bfloat16),
                        )
                        wmov = Wb_a
                    for j in range(ns):
                        nc.tensor.matmul(
                            ph[:, j, 0:COLS],
                            lhsT=xt[:, j, :],
                            rhs=wmov[:, :],
                            start=True,
                            stop=True,
                        )
                    # evict: h -> hw[k] (bf16, [n, c, s]) on ACT, score on DVE
                    nc.scalar.copy(
                        out=hw[k][:, :, s0 - lo : s0 - lo + ns],
                        in_=ph[:, 0:ns, 0:OUT].rearrange("p s c -> p c s"),
                    )
                    nc.vector.tensor_copy(
                        out=scs[k][:, :, s0 - lo : s0 - lo + ns],
                        in_=ph[:, 0:ns, OUT:COLS].rearrange("p s h -> p h s"),
                    )
                    if s0 + ns == HALF:
                        tail_half(0)
                    elif s0 + ns == S:
                        tail_half(1)

                den = smp.tile([128, H], dt.float32, tag="den")
                nc.vector.tensor_reduce(
                    out=den[:, :],
                    in_=uf[0][:, :, :],
                    axis=mybir.AxisListType.X,
                    op=mybir.AluOpType.add,
                )
                den2 = smp.tile([128, H], dt.float32, tag="den2")
                nc.vector.tensor_reduce(
                    out=den2[:, :],
                    in_=uf[1][:, :, :],
                    axis=mybir.AxisListType.X,
                    op=mybir.AluOpType.add,
                )
                nc.vector.tensor_tensor(
                    out=den[:, :],
                    in0=den[:, :],
                    in1=den2[:, :],
                    op=mybir.AluOpType.add,
                )
                rec = smp.tile([128, H], dt.float32, tag="rec")
                nc.vector.reciprocal(rec[:, :], den[:, :])
                pr = outp.tile([128, OUT], dt.float32, tag="pr")
                nc.vector.tensor_tensor(
                    out=pr[:, :],
                    in0=prh[0][:, :],
                    in1=prh[1][:, :],
                    op=mybir.AluOpType.add,
                )
                of = outp.tile([128, OUT], dt.float32, tag="of")
                o0 = pr[:, :].rearrange("p (h o) -> p h o", h=H)
                o1 = rec[:, :].unsqueeze(2)  # [p, h, 1]
                oo = of[:, :].rearrange("p (h o) -> p h o", h=H)
                o0b, o1b = broadcast_tensor_aps(o0, o1)
                nc.vector.tensor_tensor(
                    out=oo, in0=o0b, in1=o1b, op=mybir.AluOpType.mult
                )
                nc.sync.dma_start(out=od[b * BLK : (b + 1) * BLK, :], in_=of[:, :])

    nc.finalize()
    return nc


def _get(npc):
    if npc not in _BUILT:
        _BUILT[npc] = build_bass(npc)
    return _BUILT[npc]


def kernel(x, W, q, _trace=False):
    x = np.ascontiguousarray(np.asarray(x, dtype=np.float32))
    W = np.ascontiguousarray(np.asarray(W, dtype=np.float32))
    q = np.ascontiguousarray(np.asarray(q, dtype=np.float32))
    n = x.shape[0]
    npc = n // NCORES
    nc = _get(npc)

    from concourse.bass_utils import run_bass_kernel_spmd

    in_maps = [
        {"x": x[c * npc : (c + 1) * npc], "W": W, "q": q} for c in range(NCORES)
    ]
    res = run_bass_kernel_spmd(
        nc, in_maps, core_ids=list(range(NCORES)), trace=_trace
    )
    out = np.concatenate([res.results[c]["out"] for c in range(NCORES)], axis=0)
    if _trace:
        return out.astype(np.float32), res
    return out.astype(np.float32)
